# revision 18
# baseline (speedup 1.0000x reference)
"""ChannelAttention1D (SE-MLP over frame means + top-k frame gather) on 8 trn2 cores.

Math (per sample b of B=4096):
    pooled[f] = mean_d x[b, f, d]                    f in [0, 64)
    h = relu(pooled @ w1.T + b1)                     [16]
    logits = h @ w2.T + b2                           [64]  (sigmoid is monotonic -> skipped)
    idx = top_k(logits, 8).indices
    out[b, k, :] = x[b, idx[k], :]

Sharding: pure data-parallel over batch, 512 samples per core; SE weights replicated.

Per-core kernel (all shapes hardcoded):
  Phase A (per group of 128 samples): stream x as [128 (b,f)-rows, 8, 512] tiles,
  two-stage DVE reduce over D -> pooled sums laid out [f-major, sample-pair] so the
  PE can contract over F directly (even/odd sample split across partition halves).
  SE-MLP on PE (K=64 then K=16 via a PE transpose), top-8 via DVE MAX + MAX_INDEX.
  Indices are turned into global (b*64+f) row ids.
  Phase B: one indirect-DMA gather per group pulls the selected 2KB frame rows
  from HBM into SBUF keyed by a [128, 8] offset tile, then one 2MB store with a
  sample-unpermuting DRAM access pattern writes the output.

  DMA engine split: x-loads on HWDGE (sync, SP ring), output stores on HWDGE
  (scalar, ACT ring), indirect gathers on SWDGE (gpsimd).

  Built on Bacc (not raw Bass) so nc.compile() runs
  generate_event_semaphores: this walrus permits at most one sync wait per
  instruction, and that pass splits multi-waits into EventSemaphore ops.
"""

import sys

sys.path.insert(0, "/opt/trn_rl_repo")

import numpy as np

B, F, D = 4096, 64, 512
K = 8
NCORES = 8
BL = B // NCORES  # 512 samples per core
GROUPS = BL // 128  # 4
NLOAD = 8  # x-load DMAs per group
CPL = 64 // NLOAD  # pooled columns (sample pairs) per load = 8

_cache = {}


def _build_nc():
    import concourse.bass as bass
    import concourse.bacc as bacc
    import concourse.mybir as mybir
    import concourse.tile as tile
    from contextlib import ExitStack

    f32 = mybir.dt.float32
    u32 = mybir.dt.uint32
    X = mybir.AxisListType.X
    ADD = mybir.AluOpType.add

    nc = bacc.Bacc(
        "TRN2", target_bir_lowering=False, debug=False, num_devices=NCORES
    )
    x = nc.declare_dram_parameter("x", [BL, F, D], f32, isOutput=False)
    w1t = nc.declare_dram_parameter("w1t", [128, 16], f32, isOutput=False)
    w2t = nc.declare_dram_parameter("w2t", [16, 64], f32, isOutput=False)
    b1b = nc.declare_dram_parameter("b1b", [128, 16], f32, isOutput=False)
    b2b = nc.declare_dram_parameter("b2b", [128, 64], f32, isOutput=False)
    offc = nc.declare_dram_parameter("offc", [128, GROUPS], f32, isOutput=False)
    ident = nc.declare_dram_parameter("ident", [128, 128], f32, isOutput=False)
    out = nc.declare_dram_parameter("out", [BL, K, D], f32, isOutput=True)

    x_flat = x[:].rearrange("b f d -> (b f) d")  # [BL*64, 512]
    out_flat = out[:].rearrange("b k d -> (b k) d")  # [BL*8, 512]

    with ExitStack() as ctx:
        tc = ctx.enter_context(tile.TileContext(nc))
        consts = ctx.enter_context(tc.tile_pool(name="consts", bufs=1))
        xpool = ctx.enter_context(tc.tile_pool(name="xpool", bufs=6))
        small = ctx.enter_context(tc.tile_pool(name="small", bufs=4))
        psum = ctx.enter_context(tc.tile_pool(name="psum", bufs=2, space="PSUM"))
        gpool = ctx.enter_context(tc.tile_pool(name="gpool", bufs=2))

        w1t_sb = consts.tile([128, 16], f32)
        nc.sync.dma_start(out=w1t_sb[:], in_=w1t[:])
        w2t_sb = consts.tile([16, 64], f32)
        nc.sync.dma_start(out=w2t_sb[:], in_=w2t[:])
        b1b_sb = consts.tile([128, 16], f32)
        nc.sync.dma_start(out=b1b_sb[:], in_=b1b[:])
        b2b_sb = consts.tile([128, 64], f32)
        nc.sync.dma_start(out=b2b_sb[:], in_=b2b[:])
        offc_sb = consts.tile([128, GROUPS], f32)
        nc.sync.dma_start(out=offc_sb[:], in_=offc[:])
        id_sb = consts.tile([128, 128], f32)
        nc.sync.dma_start(out=id_sb[:], in_=ident[:])

        for g in range(GROUPS):
            # ---- phase A: pooled sums for 128 samples ----
            # pooled[p, t]: p = b_lo*64 + f (b_lo = sample parity), t = sample pair
            pooled = small.tile([128, 64], f32, tag="pooled")
            for i in range(NLOAD):
                xt = xpool.tile([128, CPL, D], f32, tag="xt")
                row0 = g * 8192 + i * (128 * CPL)
                nc.sync.dma_start(
                    out=xt[:],
                    in_=x_flat[row0 : row0 + 128 * CPL, :].rearrange(
                        "(c p) d -> p c d", p=128
                    ),
                )
                part = small.tile([128, CPL, 16], f32, tag="part")
                nc.vector.tensor_reduce(
                    out=part[:],
                    in_=xt[:].rearrange("p c (s t) -> p c s t", t=32),
                    axis=X,
                    op=ADD,
                )
                nc.vector.tensor_reduce(
                    out=pooled[:, i * CPL : (i + 1) * CPL], in_=part[:], axis=X, op=ADD
                )

            # ---- SE-MLP ----
            # h = relu(pooled_mean @ w1.T + b1); the 1/512 mean scale is folded
            # into w1t on the host (exact, power of two).
            h_ps = psum.tile([128, 16], f32, tag="hps")
            nc.tensor.matmul(
                out=h_ps[0:64, :],
                lhsT=pooled[0:64, :],
                rhs=w1t_sb[0:64, :],
                start=True,
                stop=True,
            )
            nc.tensor.matmul(
                out=h_ps[64:128, :],
                lhsT=pooled[64:128, :],
                rhs=w1t_sb[64:128, :],
                start=True,
                stop=True,
            )
            h_sb = small.tile([128, 16], f32, tag="hsb")
            nc.vector.tensor_add(out=h_sb[:], in0=h_ps[:], in1=b1b_sb[:])
            nc.vector.tensor_scalar_max(h_sb[:], h_sb[:], 0.0)

            ht_ps = psum.tile([16, 128], f32, tag="htps")
            nc.tensor.transpose(out=ht_ps[:], in_=h_sb[:], identity=id_sb[:])
            ht_sb = small.tile([16, 128], f32, tag="htsb")
            nc.scalar.copy(out=ht_sb[:], in_=ht_ps[:])

            lg_ps = psum.tile([128, 64], f32, tag="lgps")
            nc.tensor.matmul(
                out=lg_ps[:], lhsT=ht_sb[:], rhs=w2t_sb[:], start=True, stop=True
            )
            lg_sb = small.tile([128, 64], f32, tag="lgsb")
            nc.vector.tensor_add(out=lg_sb[:], in0=lg_ps[:], in1=b2b_sb[:])

            # ---- top-8 ----
            mx8 = small.tile([128, 8], f32, tag="mx8")
            idx8 = small.tile([128, 8], u32, tag="idx8")
            nc.vector.max(out=mx8[:], in_=lg_sb[:])
            nc.vector.max_index(out=idx8[:], in_max=mx8[:], in_values=lg_sb[:])

            # global x_flat row id = idx + (g*128 + sample(p))*64, per-partition
            # const; computed in f32 (exact, values < 2^24) since tensor_scalar
            # AP operands must be f32
            idx8f = small.tile([128, 8], f32, tag="idx8f")
            nc.vector.tensor_copy(out=idx8f[:], in_=idx8[:])
            nc.vector.tensor_scalar(
                out=idx8f[:],
                in0=idx8f[:],
                scalar1=offc_sb[:, g : g + 1],
                scalar2=None,
                op0=ADD,
            )
            grow = small.tile([128, 8], u32, tag="grow")
            nc.vector.tensor_copy(out=grow[:], in_=idx8f[:])

            # ---- phase B: gather the selected frame rows ----
            # gt[p', k, :] = x_flat[grow[p', k], :]; indirect DMA supports one
            # offset per partition, so one gather per rank k
            gt = gpool.tile([128, K, D], f32, tag="gt")
            for k in range(K):
                nc.gpsimd.indirect_dma_start(
                    out=gt[:, k, :],
                    out_offset=None,
                    in_=x_flat,
                    in_offset=bass.IndirectOffsetOnAxis(ap=grow[:, k : k + 1], axis=0),
                )
            # store, un-permuting p' = b_lo*64 + t back to sample 2t + b_lo:
            # out row (g*128 + 2t + b_lo)*8 + k = group base + t*16 + b_lo*8 + k
            # 4-dim DRAM AP [b_lo, t, k, d]; iteration order matches the SBUF
            # side's partition order p' = b_lo*64 + t
            ov = out_flat[g * 1024 : (g + 1) * 1024, :].rearrange(
                "(t two k) d -> two t k d", two=2, k=K
            )
            nc.scalar.dma_start(out=ov, in_=gt[:])

    nc.compile()
    return nc


def _consts(w1, b1, w2, b2):
    w1t = np.ascontiguousarray(w1.T.astype(np.float32)) / 512.0  # fold mean scale
    w1t_dup = np.concatenate([w1t, w1t], axis=0)  # [128, 16]
    w2t = np.ascontiguousarray(w2.T.astype(np.float32))  # [16, 64]
    b1b = np.tile(np.asarray(b1, np.float32)[None, :], (128, 1))
    b2b = np.tile(np.asarray(b2, np.float32)[None, :], (128, 1))
    p = np.arange(128)
    sample = 2 * (p % 64) + p // 64  # partition -> sample within group
    offc = (
        sample[:, None] * 64 + np.arange(GROUPS)[None, :] * (128 * 64)
    ).astype(np.float32)
    ident = np.eye(128, dtype=np.float32)
    return w1t_dup, w2t, b1b, b2b, offc, ident


def make_in_maps(x, w1, b1, w2, b2):
    x = np.asarray(x)
    w1t_dup, w2t, b1b, b2b, offc, ident = _consts(
        np.asarray(w1), np.asarray(b1), np.asarray(w2), np.asarray(b2)
    )
    in_maps = []
    for i in range(NCORES):
        in_maps.append(
            {
                "x": np.ascontiguousarray(x[i * BL : (i + 1) * BL]),
                "w1t": w1t_dup,
                "w2t": w2t,
                "b1b": b1b,
                "b2b": b2b,
                "offc": offc,
                "ident": ident,
            }
        )
    return in_maps


def build_nc():
    if "nc" not in _cache:
        _cache["nc"] = _build_nc()
    return _cache["nc"]


def kernel(x, w1, b1, w2, b2):
    from concourse.bass_utils import run_bass_kernel_spmd

    in_maps = make_in_maps(x, w1, b1, w2, b2)
    res = run_bass_kernel_spmd(build_nc(), in_maps, list(range(NCORES)))
    return np.concatenate([r["out"] for r in res.results], axis=0)


# revision 29
# speedup vs baseline: 24.0684x; 24.0684x over previous
"""ChannelAttention1D (SE-MLP over frame means + top-k frame gather) on 8 trn2 cores.

Math (per sample b of B=4096):
    pooled[f] = mean_d x[b, f, d]                    f in [0, 64)
    h = relu(pooled @ w1.T + b1)                     [16]
    logits = h @ w2.T + b2                           [64]  (sigmoid is monotonic -> skipped)
    idx = top_k(logits, 8).indices
    out[b, k, :] = x[b, idx[k], :]

Sharding: pure data-parallel over batch, 512 samples per core; SE weights replicated.

Per-core kernel (all shapes hardcoded):
  Phase A (per group of 128 samples): stream x as two 8MB [128 (b,f)-rows, 32,
  512] tiles (big DMAs = best measured HBM efficiency), two-stage DVE reduce
  over D -> pooled sums laid out [f-major, sample-pair] so the PE can contract
  over F directly (even/odd sample split across partition halves). SE-MLP on PE
  (K=64 then K=16 via a PE transpose), top-8 via DVE MAX + MAX_INDEX.
  Indices are turned into global (b*64+f) row ids.
  Phase B: 8 indirect-DMA gathers per group (one per rank k, offsets
  grow[:, k]) pull the selected 2KB frame rows from HBM into a [128, 8, 512]
  SBUF tile, then one 2MB store with a sample-unpermuting 4-dim DRAM access
  pattern writes the output.

  DMA engine split: x-loads on HWDGE (sync, SP ring), output stores on HWDGE
  (scalar, ACT ring), indirect gathers on SWDGE (gpsimd).

  Built on Bacc (not raw Bass) so nc.compile() runs
  generate_event_semaphores: this walrus permits at most one sync wait per
  instruction, and that pass splits multi-waits into EventSemaphore ops.
"""

import sys

sys.path.insert(0, "/opt/trn_rl_repo")

import numpy as np

B, F, D = 4096, 64, 512
K = 8
NCORES = 8
BL = B // NCORES  # 512 samples per core
GROUPS = BL // 128  # 4
NLOAD = 2  # x-load DMAs per group
CPL = 64 // NLOAD  # pooled columns (sample pairs) per load

_cache = {}


def _build_nc():
    import concourse.bass as bass
    import concourse.bacc as bacc
    import concourse.mybir as mybir
    import concourse.tile as tile
    from contextlib import ExitStack

    f32 = mybir.dt.float32
    u32 = mybir.dt.uint32
    X = mybir.AxisListType.X
    ADD = mybir.AluOpType.add

    nc = bacc.Bacc(
        "TRN2", target_bir_lowering=False, debug=False, num_devices=NCORES
    )
    x = nc.declare_dram_parameter("x", [BL, F, D], f32, isOutput=False)
    w1t = nc.declare_dram_parameter("w1t", [128, 16], f32, isOutput=False)
    w2t = nc.declare_dram_parameter("w2t", [16, 64], f32, isOutput=False)
    b1b = nc.declare_dram_parameter("b1b", [128, 16], f32, isOutput=False)
    b2b = nc.declare_dram_parameter("b2b", [128, 64], f32, isOutput=False)
    offc = nc.declare_dram_parameter("offc", [128, GROUPS], f32, isOutput=False)
    ident = nc.declare_dram_parameter("ident", [128, 128], f32, isOutput=False)
    out = nc.declare_dram_parameter("out", [BL, K, D], f32, isOutput=True)

    x_flat = x[:].rearrange("b f d -> (b f) d")  # [BL*64, 512]
    out_flat = out[:].rearrange("b k d -> (b k) d")  # [BL*8, 512]

    with ExitStack() as ctx:
        tc = ctx.enter_context(tile.TileContext(nc))
        consts = ctx.enter_context(tc.tile_pool(name="consts", bufs=1))
        xpool = ctx.enter_context(tc.tile_pool(name="xpool", bufs=2))
        small = ctx.enter_context(tc.tile_pool(name="small", bufs=4))
        psum = ctx.enter_context(tc.tile_pool(name="psum", bufs=2, space="PSUM"))
        gpool = ctx.enter_context(tc.tile_pool(name="gpool", bufs=2))

        # constants via SWDGE so the HWDGE rings start streaming x immediately
        w1t_sb = consts.tile([128, 16], f32)
        nc.gpsimd.dma_start(out=w1t_sb[:], in_=w1t[:])
        w2t_sb = consts.tile([16, 64], f32)
        nc.gpsimd.dma_start(out=w2t_sb[:], in_=w2t[:])
        b1b_sb = consts.tile([128, 16], f32)
        nc.gpsimd.dma_start(out=b1b_sb[:], in_=b1b[:])
        b2b_sb = consts.tile([128, 64], f32)
        nc.gpsimd.dma_start(out=b2b_sb[:], in_=b2b[:])
        offc_sb = consts.tile([128, GROUPS], f32)
        nc.gpsimd.dma_start(out=offc_sb[:], in_=offc[:])
        id_sb = consts.tile([128, 128], f32)
        nc.gpsimd.dma_start(out=id_sb[:], in_=ident[:])

        for g in range(GROUPS):
            # ---- phase A: pooled sums for 128 samples ----
            # pooled[p, t]: p = b_lo*64 + f (b_lo = sample parity), t = sample pair
            pooled = small.tile([128, 64], f32, tag="pooled")
            for i in range(NLOAD):
                xt = xpool.tile([128, CPL, D], f32, tag="xt")
                row0 = g * 8192 + i * (128 * CPL)
                nc.sync.dma_start(
                    out=xt[:],
                    in_=x_flat[row0 : row0 + 128 * CPL, :].rearrange(
                        "(c p) d -> p c d", p=128
                    ),
                )
                part = small.tile([128, CPL, 16], f32, tag="part")
                nc.vector.tensor_reduce(
                    out=part[:],
                    in_=xt[:].rearrange("p c (s t) -> p c s t", t=32),
                    axis=X,
                    op=ADD,
                )
                nc.vector.tensor_reduce(
                    out=pooled[:, i * CPL : (i + 1) * CPL], in_=part[:], axis=X, op=ADD
                )

            # ---- SE-MLP ----
            # h = relu(pooled_mean @ w1.T + b1); the 1/512 mean scale is folded
            # into w1t on the host (exact, power of two).
            h_ps = psum.tile([128, 16], f32, tag="hps")
            nc.tensor.matmul(
                out=h_ps[0:64, :],
                lhsT=pooled[0:64, :],
                rhs=w1t_sb[0:64, :],
                start=True,
                stop=True,
            )
            nc.tensor.matmul(
                out=h_ps[64:128, :],
                lhsT=pooled[64:128, :],
                rhs=w1t_sb[64:128, :],
                start=True,
                stop=True,
            )
            h_sb = small.tile([128, 16], f32, tag="hsb")
            nc.vector.tensor_add(out=h_sb[:], in0=h_ps[:], in1=b1b_sb[:])
            nc.vector.tensor_scalar_max(h_sb[:], h_sb[:], 0.0)

            ht_ps = psum.tile([16, 128], f32, tag="htps")
            nc.tensor.transpose(out=ht_ps[:], in_=h_sb[:], identity=id_sb[:])
            ht_sb = small.tile([16, 128], f32, tag="htsb")
            nc.scalar.copy(out=ht_sb[:], in_=ht_ps[:])

            lg_ps = psum.tile([128, 64], f32, tag="lgps")
            nc.tensor.matmul(
                out=lg_ps[:], lhsT=ht_sb[:], rhs=w2t_sb[:], start=True, stop=True
            )
            lg_sb = small.tile([128, 64], f32, tag="lgsb")
            nc.vector.tensor_add(out=lg_sb[:], in0=lg_ps[:], in1=b2b_sb[:])

            # ---- top-8 ----
            mx8 = small.tile([128, 8], f32, tag="mx8")
            idx8 = small.tile([128, 8], u32, tag="idx8")
            nc.vector.max(out=mx8[:], in_=lg_sb[:])
            nc.vector.max_index(out=idx8[:], in_max=mx8[:], in_values=lg_sb[:])

            # global x_flat row id = idx + (g*128 + sample(p))*64, per-partition
            # const; computed in f32 (exact, values < 2^24) since tensor_scalar
            # AP operands must be f32
            idx8f = small.tile([128, 8], f32, tag="idx8f")
            nc.vector.tensor_copy(out=idx8f[:], in_=idx8[:])
            nc.vector.tensor_scalar(
                out=idx8f[:],
                in0=idx8f[:],
                scalar1=offc_sb[:, g : g + 1],
                scalar2=None,
                op0=ADD,
            )
            grow = small.tile([128, 8], u32, tag="grow")
            nc.vector.tensor_copy(out=grow[:], in_=idx8f[:])

            # ---- phase B: gather the selected frame rows ----
            # gt[p', k, :] = x_flat[grow[p', k], :]; indirect DMA supports one
            # offset per partition, so one gather per rank k
            gt = gpool.tile([128, K, D], f32, tag="gt")
            for k in range(K):
                nc.gpsimd.indirect_dma_start(
                    out=gt[:, k, :],
                    out_offset=None,
                    in_=x_flat,
                    in_offset=bass.IndirectOffsetOnAxis(ap=grow[:, k : k + 1], axis=0),
                )
            # store, un-permuting p' = b_lo*64 + t back to sample 2t + b_lo:
            # out row (g*128 + 2t + b_lo)*8 + k = group base + t*16 + b_lo*8 + k
            # 4-dim DRAM AP [b_lo, t, k, d]; iteration order matches the SBUF
            # side's partition order p' = b_lo*64 + t
            ov = out_flat[g * 1024 : (g + 1) * 1024, :].rearrange(
                "(t two k) d -> two t k d", two=2, k=K
            )
            nc.scalar.dma_start(out=ov, in_=gt[:])

    nc.compile()
    return nc


def _consts(w1, b1, w2, b2):
    w1t = np.ascontiguousarray(w1.T.astype(np.float32)) / 512.0  # fold mean scale
    w1t_dup = np.concatenate([w1t, w1t], axis=0)  # [128, 16]
    w2t = np.ascontiguousarray(w2.T.astype(np.float32))  # [16, 64]
    b1b = np.tile(np.asarray(b1, np.float32)[None, :], (128, 1))
    b2b = np.tile(np.asarray(b2, np.float32)[None, :], (128, 1))
    p = np.arange(128)
    sample = 2 * (p % 64) + p // 64  # partition -> sample within group
    offc = (
        sample[:, None] * 64 + np.arange(GROUPS)[None, :] * (128 * 64)
    ).astype(np.float32)
    ident = np.eye(128, dtype=np.float32)
    return w1t_dup, w2t, b1b, b2b, offc, ident


def make_in_maps(x, w1, b1, w2, b2):
    x = np.asarray(x)
    w1t_dup, w2t, b1b, b2b, offc, ident = _consts(
        np.asarray(w1), np.asarray(b1), np.asarray(w2), np.asarray(b2)
    )
    in_maps = []
    for i in range(NCORES):
        in_maps.append(
            {
                "x": np.ascontiguousarray(x[i * BL : (i + 1) * BL]),
                "w1t": w1t_dup,
                "w2t": w2t,
                "b1b": b1b,
                "b2b": b2b,
                "offc": offc,
                "ident": ident,
            }
        )
    return in_maps


def build_nc():
    if "nc" not in _cache:
        _cache["nc"] = _build_nc()
    return _cache["nc"]


def kernel(x, w1, b1, w2, b2):
    from concourse.bass_utils import run_bass_kernel_spmd

    in_maps = make_in_maps(x, w1, b1, w2, b2)
    res = run_bass_kernel_spmd(build_nc(), in_maps, list(range(NCORES)))
    return np.concatenate([r["out"] for r in res.results], axis=0)


# revision 30
# speedup vs baseline: 29.1407x; 1.2107x over previous
"""ChannelAttention1D (SE-MLP over frame means + top-k frame gather) on 8 trn2 cores.

Math (per sample b of B=4096):
    pooled[f] = mean_d x[b, f, d]                    f in [0, 64)
    h = relu(pooled @ w1.T + b1)                     [16]
    logits = h @ w2.T + b2                           [64]  (sigmoid is monotonic -> skipped)
    idx = top_k(logits, 8).indices
    out[b, k, :] = x[b, idx[k], :]

Sharding: pure data-parallel over batch, 512 samples per core; SE weights replicated.

Per-core kernel (all shapes hardcoded):
  Phase A (per group of 128 samples): stream x as two 8MB [128 (b,f)-rows, 32,
  512] tiles (big DMAs = best measured HBM efficiency), two-stage DVE reduce
  over D -> pooled sums laid out [f-major, sample-pair] so the PE can contract
  over F directly (even/odd sample split across partition halves). SE-MLP on PE
  (K=64 then K=16 via a PE transpose), top-8 via DVE MAX + MAX_INDEX.
  Indices are turned into global (b*64+f) row ids.
  Phase B: 8 indirect-DMA gathers per group (one per rank k, offsets
  grow[:, k]) pull the selected 2KB frame rows from HBM into a [128, 8, 512]
  SBUF tile, then one 2MB store with a sample-unpermuting 4-dim DRAM access
  pattern writes the output.

  DMA engine split: x-loads on HWDGE (sync, SP ring), output stores on HWDGE
  (scalar, ACT ring), indirect gathers on SWDGE (gpsimd).

  Built on Bacc (not raw Bass) so nc.compile() runs
  generate_event_semaphores: this walrus permits at most one sync wait per
  instruction, and that pass splits multi-waits into EventSemaphore ops.
"""

import sys

sys.path.insert(0, "/opt/trn_rl_repo")

import numpy as np

B, F, D = 4096, 64, 512
K = 8
NCORES = 8
BL = B // NCORES  # 512 samples per core
GROUPS = BL // 128  # 4
NLOAD = 2  # x-load DMAs per group
CPL = 64 // NLOAD  # pooled columns (sample pairs) per load

_cache = {}


def _build_nc():
    import concourse.bass as bass
    import concourse.bacc as bacc
    import concourse.mybir as mybir
    import concourse.tile as tile
    from contextlib import ExitStack

    f32 = mybir.dt.float32
    u32 = mybir.dt.uint32
    X = mybir.AxisListType.X
    ADD = mybir.AluOpType.add

    nc = bacc.Bacc(
        "TRN2", target_bir_lowering=False, debug=False, num_devices=NCORES
    )
    x = nc.declare_dram_parameter("x", [BL, F, D], f32, isOutput=False)
    w1t = nc.declare_dram_parameter("w1t", [128, 16], f32, isOutput=False)
    w2t = nc.declare_dram_parameter("w2t", [16, 64], f32, isOutput=False)
    b1b = nc.declare_dram_parameter("b1b", [128, 16], f32, isOutput=False)
    b2b = nc.declare_dram_parameter("b2b", [128, 64], f32, isOutput=False)
    offc = nc.declare_dram_parameter("offc", [128, GROUPS], f32, isOutput=False)
    ident = nc.declare_dram_parameter("ident", [128, 128], f32, isOutput=False)
    out = nc.declare_dram_parameter("out", [BL, K, D], f32, isOutput=True)

    x_flat = x[:].rearrange("b f d -> (b f) d")  # [BL*64, 512]
    out_flat = out[:].rearrange("b k d -> (b k) d")  # [BL*8, 512]

    with ExitStack() as ctx:
        tc = ctx.enter_context(tile.TileContext(nc))
        consts = ctx.enter_context(tc.tile_pool(name="consts", bufs=1))
        xpool = ctx.enter_context(tc.tile_pool(name="xpool", bufs=2))
        small = ctx.enter_context(tc.tile_pool(name="small", bufs=4))
        psum = ctx.enter_context(tc.tile_pool(name="psum", bufs=2, space="PSUM"))
        gpool = ctx.enter_context(tc.tile_pool(name="gpool", bufs=2))

        # constants via SWDGE so the HWDGE rings start streaming x immediately
        w1t_sb = consts.tile([128, 16], f32)
        nc.gpsimd.dma_start(out=w1t_sb[:], in_=w1t[:])
        w2t_sb = consts.tile([16, 64], f32)
        nc.gpsimd.dma_start(out=w2t_sb[:], in_=w2t[:])
        b1b_sb = consts.tile([128, 16], f32)
        nc.gpsimd.dma_start(out=b1b_sb[:], in_=b1b[:])
        b2b_sb = consts.tile([128, 64], f32)
        nc.gpsimd.dma_start(out=b2b_sb[:], in_=b2b[:])
        offc_sb = consts.tile([128, GROUPS], f32)
        nc.gpsimd.dma_start(out=offc_sb[:], in_=offc[:])
        id_sb = consts.tile([128, 128], f32)
        nc.gpsimd.dma_start(out=id_sb[:], in_=ident[:])

        for g in range(GROUPS):
            # ---- phase A: pooled sums for 128 samples ----
            # pooled[p, t]: p = b_lo*64 + f (b_lo = sample parity), t = sample pair
            pooled = small.tile([128, 64], f32, tag="pooled")
            for i in range(NLOAD):
                xt = xpool.tile([128, CPL, D], f32, tag="xt")
                row0 = g * 8192 + i * (128 * CPL)
                nc.sync.dma_start(
                    out=xt[:],
                    in_=x_flat[row0 : row0 + 128 * CPL, :].rearrange(
                        "(c p) d -> p c d", p=128
                    ),
                )
                part = small.tile([128, CPL, 16], f32, tag="part")
                nc.vector.tensor_reduce(
                    out=part[:],
                    in_=xt[:].rearrange("p c (s t) -> p c s t", t=32),
                    axis=X,
                    op=ADD,
                )
                nc.vector.tensor_reduce(
                    out=pooled[:, i * CPL : (i + 1) * CPL], in_=part[:], axis=X, op=ADD
                )

            # ---- SE-MLP ----
            # h = relu(pooled_mean @ w1.T + b1); the 1/512 mean scale is folded
            # into w1t on the host (exact, power of two).
            h_ps = psum.tile([128, 16], f32, tag="hps")
            nc.tensor.matmul(
                out=h_ps[0:64, :],
                lhsT=pooled[0:64, :],
                rhs=w1t_sb[0:64, :],
                start=True,
                stop=True,
            )
            nc.tensor.matmul(
                out=h_ps[64:128, :],
                lhsT=pooled[64:128, :],
                rhs=w1t_sb[64:128, :],
                start=True,
                stop=True,
            )
            h_sb = small.tile([128, 16], f32, tag="hsb")
            nc.vector.tensor_add(out=h_sb[:], in0=h_ps[:], in1=b1b_sb[:])
            nc.vector.tensor_scalar_max(h_sb[:], h_sb[:], 0.0)

            ht_ps = psum.tile([16, 128], f32, tag="htps")
            nc.tensor.transpose(out=ht_ps[:], in_=h_sb[:], identity=id_sb[:])
            ht_sb = small.tile([16, 128], f32, tag="htsb")
            nc.scalar.copy(out=ht_sb[:], in_=ht_ps[:])

            lg_ps = psum.tile([128, 64], f32, tag="lgps")
            nc.tensor.matmul(
                out=lg_ps[:], lhsT=ht_sb[:], rhs=w2t_sb[:], start=True, stop=True
            )
            lg_sb = small.tile([128, 64], f32, tag="lgsb")
            nc.vector.tensor_add(out=lg_sb[:], in0=lg_ps[:], in1=b2b_sb[:])

            # ---- top-8 ----
            mx8 = small.tile([128, 8], f32, tag="mx8")
            idx8 = small.tile([128, 8], u32, tag="idx8")
            nc.vector.max(out=mx8[:], in_=lg_sb[:])
            nc.vector.max_index(out=idx8[:], in_max=mx8[:], in_values=lg_sb[:])

            # global x_flat row id = idx + (g*128 + sample(p))*64, per-partition
            # const; computed in f32 (exact, values < 2^24) since tensor_scalar
            # AP operands must be f32
            idx8f = small.tile([128, 8], f32, tag="idx8f")
            nc.vector.tensor_copy(out=idx8f[:], in_=idx8[:])
            nc.vector.tensor_scalar(
                out=idx8f[:],
                in0=idx8f[:],
                scalar1=offc_sb[:, g : g + 1],
                scalar2=None,
                op0=ADD,
            )
            grow = small.tile([128, 8], u32, tag="grow")
            nc.vector.tensor_copy(out=grow[:], in_=idx8f[:])

            # ---- phase B: gather the selected frame rows ----
            # gt[p', k, :] = x_flat[grow[p', k], :]; indirect DMA supports one
            # offset per partition, so one gather per rank k
            gt = gpool.tile([128, K, D], f32, tag="gt")
            for k in range(K):
                nc.gpsimd.indirect_dma_start(
                    out=gt[:, k, :],
                    out_offset=None,
                    in_=x_flat,
                    in_offset=bass.IndirectOffsetOnAxis(ap=grow[:, k : k + 1], axis=0),
                )
            # store, un-permuting p' = b_lo*64 + t back to sample 2t + b_lo:
            # out row (g*128 + 2t + b_lo)*8 + k = group base + t*16 + b_lo*8 + k
            # 4-dim DRAM AP [b_lo, t, k, d]; iteration order matches the SBUF
            # side's partition order p' = b_lo*64 + t
            ov = out_flat[g * 1024 : (g + 1) * 1024, :].rearrange(
                "(t two k) d -> two t k d", two=2, k=K
            )
            nc.scalar.dma_start(out=ov, in_=gt[:])

    nc.compile()
    return nc


def _consts(w1, b1, w2, b2):
    w1t = np.ascontiguousarray(w1.T.astype(np.float32)) / 512.0  # fold mean scale
    w1t_dup = np.concatenate([w1t, w1t], axis=0)  # [128, 16]
    w2t = np.ascontiguousarray(w2.T.astype(np.float32))  # [16, 64]
    b1b = np.tile(np.asarray(b1, np.float32)[None, :], (128, 1))
    b2b = np.tile(np.asarray(b2, np.float32)[None, :], (128, 1))
    p = np.arange(128)
    sample = 2 * (p % 64) + p // 64  # partition -> sample within group
    offc = (
        sample[:, None] * 64 + np.arange(GROUPS)[None, :] * (128 * 64)
    ).astype(np.float32)
    ident = np.eye(128, dtype=np.float32)
    return w1t_dup, w2t, b1b, b2b, offc, ident


def make_in_maps(x, w1, b1, w2, b2):
    x = np.asarray(x)
    w1t_dup, w2t, b1b, b2b, offc, ident = _consts(
        np.asarray(w1), np.asarray(b1), np.asarray(w2), np.asarray(b2)
    )
    in_maps = []
    for i in range(NCORES):
        in_maps.append(
            {
                "x": np.ascontiguousarray(x[i * BL : (i + 1) * BL]),
                "w1t": w1t_dup,
                "w2t": w2t,
                "b1b": b1b,
                "b2b": b2b,
                "offc": offc,
                "ident": ident,
            }
        )
    return in_maps


def build_nc():
    if "nc" not in _cache:
        _cache["nc"] = _build_nc()
    return _cache["nc"]


def kernel(x, w1, b1, w2, b2):
    import os

    # the NTFF trace hook (antenv.axon_hooks) doesn't exist in this container;
    # make sure an inherited BASS_TRACE can't route us onto that path
    os.environ["BASS_NEVER_TRACE"] = "1"
    from concourse.bass_utils import run_bass_kernel_spmd

    in_maps = make_in_maps(x, w1, b1, w2, b2)
    res = run_bass_kernel_spmd(build_nc(), in_maps, list(range(NCORES)))
    return np.concatenate([r["out"] for r in res.results], axis=0)


# revision 33
# speedup vs baseline: 94.6950x; 3.2496x over previous
"""ChannelAttention1D (SE-MLP over frame means + top-k frame gather) on 8 trn2 cores.

Math (per sample b of B=4096):
    pooled[f] = mean_d x[b, f, d]                    f in [0, 64)
    h = relu(pooled @ w1.T + b1)                     [16]
    logits = h @ w2.T + b2                           [64]  (sigmoid is monotonic -> skipped)
    idx = top_k(logits, 8).indices
    out[b, k, :] = x[b, idx[k], :]

Sharding: pure data-parallel over batch, 512 samples per core; SE weights replicated.

Per-core kernel (all shapes hardcoded), sample-major layout throughout — one
sample per SBUF partition, so the whole compute chain lives on the DVE and no
cross-engine (PE/ACT) hops sit on the critical path. An earlier PE/ACT-based
variant (f-major pooled, PE matmuls + transpose) consistently measured slower:
the Tile scheduler batches same-kind PE ops across groups, which serialized
every group's topk/gather behind the last group's reduce (a ~60 us dependency
tail in the cost-model timeline, and a large same-session A/B gap on HW).

  Per group of 128 samples:
    - two 8MB HWDGE loads x[g*128:(g+1)*128, 32-frame slice, :] -> [128, 32,
      512] tiles (64KB contiguous per partition)
    - two-stage DVE reduce over D (inner 32, then 16) -> pooled sums [128, 64]
    - SE-MLP on DVE via broadcast tensor_tensor multiplies against
      host-replicated weight rows + segmented tensor_reduce:
        h = relu(sum_f pooled*w1bc + b1), logits = sum_r h*w2bc + b2
      (the 1/512 mean scale is folded into w1bc on the host; exact, power of 2)
    - top-8 via the DVE MAX + MAX_INDEX instructions (K=8 = HW width)
    - global row ids grow[p, k] = (g*128+p)*64 + idx, computed in f32 (exact)
    - 8 indirect SWDGE gathers (one per rank k, one offset per partition) pull
      the selected 2KB frame rows from HBM into gt [128, 8, 512]
    - one contiguous 2MB HWDGE store gt -> out[g*128:(g+1)*128]

  DMA split: x-loads on the sync (SP) HWDGE ring, output stores on the scalar
  (ACT) ring, constants + indirect gathers on SWDGE (gpsimd).

  Built on Bacc (not raw Bass) so nc.compile() runs generate_event_semaphores:
  this walrus permits at most one sync wait per instruction, and that pass
  splits multi-waits into EventSemaphore ops.
"""

import sys

sys.path.insert(0, "/opt/trn_rl_repo")

import numpy as np

B, F, D = 4096, 64, 512
K = 8
NCORES = 8
BL = B // NCORES  # 512 samples per core
GROUPS = BL // 128  # 4
NLOAD = 2  # x-load DMAs per group
FPL = F // NLOAD  # frames per load = 32
R = 16  # SE bottleneck width

_cache = {}


def _build_nc():
    import concourse.bass as bass
    import concourse.bacc as bacc
    import concourse.mybir as mybir
    import concourse.tile as tile
    from contextlib import ExitStack

    f32 = mybir.dt.float32
    u32 = mybir.dt.uint32
    X = mybir.AxisListType.X
    ADD = mybir.AluOpType.add
    MULT = mybir.AluOpType.mult

    nc = bacc.Bacc(
        "TRN2", target_bir_lowering=False, debug=False, num_devices=NCORES
    )
    x = nc.declare_dram_parameter("x", [BL, F, D], f32, isOutput=False)
    w1bc = nc.declare_dram_parameter("w1bc", [128, R * F], f32, isOutput=False)
    w2bc = nc.declare_dram_parameter("w2bc", [128, F * R], f32, isOutput=False)
    b1b = nc.declare_dram_parameter("b1b", [128, R], f32, isOutput=False)
    b2b = nc.declare_dram_parameter("b2b", [128, F], f32, isOutput=False)
    offc = nc.declare_dram_parameter("offc", [128, GROUPS], f32, isOutput=False)
    out = nc.declare_dram_parameter("out", [BL, K, D], f32, isOutput=True)

    x_ap = x[:]
    x_flat = x_ap.rearrange("b f d -> (b f) d")  # [BL*64, 512]
    out_ap = out[:]

    def bcast_mid(ap, n):
        # [P, A] -> [P, n, A] with a stride-0 broadcast middle dim
        return bass.AP(
            tensor=ap.tensor, offset=ap.offset, ap=[ap.ap[0], [0, n], *ap.ap[1:]]
        )

    with ExitStack() as ctx:
        tc = ctx.enter_context(tile.TileContext(nc))
        consts = ctx.enter_context(tc.tile_pool(name="consts", bufs=1))
        xpool = ctx.enter_context(tc.tile_pool(name="xpool", bufs=2))
        small = ctx.enter_context(tc.tile_pool(name="small", bufs=2))
        gpool = ctx.enter_context(tc.tile_pool(name="gpool", bufs=2))

        w1bc_sb = consts.tile([128, R * F], f32)
        nc.gpsimd.dma_start(out=w1bc_sb[:], in_=w1bc[:])
        w2bc_sb = consts.tile([128, F * R], f32)
        nc.gpsimd.dma_start(out=w2bc_sb[:], in_=w2bc[:])
        b1b_sb = consts.tile([128, R], f32)
        nc.gpsimd.dma_start(out=b1b_sb[:], in_=b1b[:])
        b2b_sb = consts.tile([128, F], f32)
        nc.gpsimd.dma_start(out=b2b_sb[:], in_=b2b[:])
        offc_sb = consts.tile([128, GROUPS], f32)
        nc.gpsimd.dma_start(out=offc_sb[:], in_=offc[:])

        for g in range(GROUPS):
            s0 = g * 128
            # ---- pooled sums [sample, frame] ----
            pooled = small.tile([128, F], f32, tag="pooled")
            for i in range(NLOAD):
                xt = xpool.tile([128, FPL, D], f32, tag="xt")
                nc.sync.dma_start(
                    out=xt[:],
                    in_=x_ap[s0 : s0 + 128, i * FPL : (i + 1) * FPL, :],
                )
                part = small.tile([128, FPL, 16], f32, tag="part")
                nc.vector.tensor_reduce(
                    out=part[:],
                    in_=xt[:].rearrange("p c (s t) -> p c s t", t=32),
                    axis=X,
                    op=ADD,
                )
                nc.vector.tensor_reduce(
                    out=pooled[:, i * FPL : (i + 1) * FPL], in_=part[:], axis=X, op=ADD
                )

            # ---- SE-MLP, entirely on DVE ----
            # h[p, r] = sum_f pooled[p, f] * w1bc[p, r*64+f]  (w1bc pre-scaled)
            tmp1 = small.tile([128, R, F], f32, tag="tmp1")
            nc.vector.tensor_tensor(
                out=tmp1[:],
                in0=bcast_mid(pooled[:], R),
                in1=w1bc_sb[:].rearrange("p (r f) -> p r f", f=F),
                op=MULT,
            )
            hr = small.tile([128, R], f32, tag="hr")
            nc.vector.tensor_reduce(out=hr[:], in_=tmp1[:], axis=X, op=ADD)
            nc.vector.tensor_add(out=hr[:], in0=hr[:], in1=b1b_sb[:])
            nc.vector.tensor_scalar_max(hr[:], hr[:], 0.0)

            # logits[p, f] = sum_r h[p, r] * w2bc[p, f*16+r]
            tmp2 = small.tile([128, F, R], f32, tag="tmp2")
            nc.vector.tensor_tensor(
                out=tmp2[:],
                in0=bcast_mid(hr[:], F),
                in1=w2bc_sb[:].rearrange("p (f r) -> p f r", r=R),
                op=MULT,
            )
            lg = small.tile([128, F], f32, tag="lg")
            nc.vector.tensor_reduce(out=lg[:], in_=tmp2[:], axis=X, op=ADD)
            nc.vector.tensor_add(out=lg[:], in0=lg[:], in1=b2b_sb[:])

            # ---- top-8 ----
            mx8 = small.tile([128, K], f32, tag="mx8")
            idx8 = small.tile([128, K], u32, tag="idx8")
            nc.vector.max(out=mx8[:], in_=lg[:])
            nc.vector.max_index(out=idx8[:], in_max=mx8[:], in_values=lg[:])

            # global x_flat row id = (g*128 + p)*64 + idx; f32 math is exact
            # for values < 2^24 and tensor_scalar AP operands must be f32
            idx8f = small.tile([128, K], f32, tag="idx8f")
            nc.vector.tensor_copy(out=idx8f[:], in_=idx8[:])
            nc.vector.tensor_scalar(
                out=idx8f[:],
                in0=idx8f[:],
                scalar1=offc_sb[:, g : g + 1],
                scalar2=None,
                op0=ADD,
            )
            grow = small.tile([128, K], u32, tag="grow")
            nc.vector.tensor_copy(out=grow[:], in_=idx8f[:])

            # ---- gather + store ----
            gt = gpool.tile([128, K, D], f32, tag="gt")
            for k in range(K):
                nc.gpsimd.indirect_dma_start(
                    out=gt[:, k, :],
                    out_offset=None,
                    in_=x_flat,
                    in_offset=bass.IndirectOffsetOnAxis(ap=grow[:, k : k + 1], axis=0),
                )
            nc.scalar.dma_start(out=out_ap[s0 : s0 + 128, :, :], in_=gt[:])

    nc.compile()
    return nc


def _consts(w1, b1, w2, b2):
    w1s = (np.asarray(w1, np.float32) / 512.0).reshape(1, -1)  # [1, 16*64], r-major
    w2f = np.asarray(w2, np.float32).reshape(1, -1)  # [1, 64*16], f-major
    w1bc = np.ascontiguousarray(np.tile(w1s, (128, 1)))
    w2bc = np.ascontiguousarray(np.tile(w2f, (128, 1)))
    b1b = np.tile(np.asarray(b1, np.float32)[None, :], (128, 1))
    b2b = np.tile(np.asarray(b2, np.float32)[None, :], (128, 1))
    p = np.arange(128)
    offc = (
        (p[:, None] + np.arange(GROUPS)[None, :] * 128) * 64
    ).astype(np.float32)
    return w1bc, w2bc, b1b, b2b, offc


def make_in_maps(x, w1, b1, w2, b2):
    x = np.asarray(x)
    w1bc, w2bc, b1b, b2b, offc = _consts(
        np.asarray(w1), np.asarray(b1), np.asarray(w2), np.asarray(b2)
    )
    in_maps = []
    for i in range(NCORES):
        in_maps.append(
            {
                "x": np.ascontiguousarray(x[i * BL : (i + 1) * BL]),
                "w1bc": w1bc,
                "w2bc": w2bc,
                "b1b": b1b,
                "b2b": b2b,
                "offc": offc,
            }
        )
    return in_maps


def build_nc():
    if "nc" not in _cache:
        _cache["nc"] = _build_nc()
    return _cache["nc"]


def kernel(x, w1, b1, w2, b2):
    import os

    # the NTFF trace hook (antenv.axon_hooks) doesn't exist in this container;
    # make sure an inherited BASS_TRACE can't route us onto that path
    os.environ["BASS_NEVER_TRACE"] = "1"
    from concourse.bass_utils import run_bass_kernel_spmd

    in_maps = make_in_maps(x, w1, b1, w2, b2)
    res = run_bass_kernel_spmd(build_nc(), in_maps, list(range(NCORES)))
    return np.concatenate([r["out"] for r in res.results], axis=0)
